# revision 8
# baseline (speedup 1.0000x reference)
"""Trainium2 Bass kernel for nn_ContentionPredictor (embedding_lookup, memory-bound).

Computes, for each row of x (B=131072, D=1029):
    task_id = argmax(x[:, 4:1028]); ce = mean(contention[task_id]) * x[:, 0]
    out = relu(relu(relu([x[:,0:4], ce, x[:,4:1028]] @ w1 + b1) @ w2 + b2) @ w3 + b3)

Strategy (data-parallel over 8 NeuronCores, 16384 rows each):
  - SWDGE cast-DMA loads x fp32->bf16 in natural layout; HWDGE xbar DMA-transpose
    produces the K-on-partitions operand for the PE matmuls (w1 stationary).
  - argmax/gather is done without indices: DMA CCE-max tree folds the onehot
    block 1024->128 cols, DVE reduce_max gives the row max; a dual-op
    tensor_scalar builds mask*(density); multiplying by the broadcast row-means
    of contention and summing (DMA CCE-add folds + ACT accumulate) yields
    ce = density * rowmean[argmax] exactly (mask is one-hot).
  - MLP runs transposed (h1T = W.T @ xT) with relu+bias fused into the ACT
    PSUM evictions; ce enters h1 as a rank-1 PE update via a PE-transposed ce.
"""

import math
import os
from contextlib import ExitStack
from functools import lru_cache

import numpy as np
import ml_dtypes

import concourse.bass as bass
import concourse.bacc as bacc
import concourse.tile as tile
from concourse import mybir
from concourse._compat import with_exitstack

BF16 = mybir.dt.bfloat16
F32 = mybir.dt.float32

B = 131072
D = 1029
T = 1024
H1, H2 = 64, 32
N_CORES = 8
ROWS_PER_CORE = B // N_CORES  # 16384
TILE = 512                    # rows per pipeline tile
SUBS = TILE // 128            # 4 partition sub-blocks per tile

AX = mybir.AxisListType
ALU = mybir.AluOpType
AF = mybir.ActivationFunctionType


def make_body(rows_per_core):
    n_tiles = rows_per_core // TILE
    assert n_tiles * TILE == rows_per_core

    @with_exitstack
    def body(ctx: ExitStack, tc: tile.TileContext, outs, ins):
        nc = tc.nc
        x = ins["x"]
        out = outs["out"]

        singles = ctx.enter_context(tc.tile_pool(name="singles", bufs=1))
        xp = ctx.enter_context(tc.tile_pool(name="xp", bufs=2))
        xtp = ctx.enter_context(tc.tile_pool(name="xtp", bufs=2))
        app = ctx.enter_context(tc.tile_pool(name="app", bufs=2))
        sm = ctx.enter_context(tc.tile_pool(name="sm", bufs=3))
        ps_h1 = ctx.enter_context(tc.tile_pool(name="ps_h1", bufs=2, space="PSUM"))
        ps_sm = ctx.enter_context(tc.tile_pool(name="ps_sm", bufs=2, space="PSUM"))

        # --- constants (loaded once) ---
        rm_sb = singles.tile([128, T], BF16)
        nc.sync.dma_start(out=rm_sb, in_=ins["rm_b"])
        wbig_sb = singles.tile([128, 8, H1], BF16)
        nc.sync.dma_start(out=wbig_sb, in_=ins["wbig"].rearrange("c k m -> k c m"))
        w9_sb = singles.tile([128, H1], BF16)
        nc.sync.dma_start(out=w9_sb, in_=ins["w9"])
        w14_sb = singles.tile([1, H1], BF16)
        nc.sync.dma_start(out=w14_sb, in_=ins["w14"])
        w2_sb = singles.tile([H1, H2], BF16)
        nc.sync.dma_start(out=w2_sb, in_=ins["w2"])
        w3_sb = singles.tile([H2, 1], BF16)
        nc.sync.dma_start(out=w3_sb, in_=ins["w3"])
        b1_sb = singles.tile([H1, 1], F32)
        nc.sync.dma_start(out=b1_sb, in_=ins["b1"])
        b2_sb = singles.tile([H2, 1], F32)
        nc.sync.dma_start(out=b2_sb, in_=ins["b2"])
        b3_sb = singles.tile([1, 1], F32)
        nc.sync.dma_start(out=b3_sb, in_=ins["b3"])
        id_sb = singles.tile([128, 128], F32)
        nc.sync.dma_start(out=id_sb, in_=ins["ident"])

        # rm broadcast across the sub-block (mid) dim: (128, SUBS, 1024), step 0
        rm_ap = rm_sb[:, :]
        rm_bc = bass.AP(
            tensor=rm_ap.tensor,
            offset=rm_ap.offset,
            ap=[rm_ap.ap[0], [0, SUBS], rm_ap.ap[1]],
        )

        for t in range(n_tiles):
            # ---- load tile (cast fp32 -> bf16), natural layout ----
            xr = x[t * TILE:(t + 1) * TILE, 0:1028].rearrange(
                "(s p) c -> p s c", p=128
            )
            xn = xp.tile([128, SUBS, 1028], BF16, tag="xn")
            nc.gpsimd.dma_start(out=xn, in_=xr)

            # ---- transpose for matmul: cols 0:1024 (8 chunks) + tail 900:1028 ----
            xt = xtp.tile([128, SUBS, 8, 128], BF16, tag="xt")
            xt9 = xtp.tile([128, SUBS, 128], BF16, tag="xt9")
            for s in range(SUBS):
                nc.sync.dma_start(out=xt[:, s, :, :], in_=xn[:, s, 0:1024],
                                  transpose=True)
                nc.sync.dma_start(out=xt9[:, s, :], in_=xn[:, s, 900:1028],
                                  transpose=True)

            # ---- density (col 0) as fp32 per-partition scalars ----
            dens = sm.tile([128, SUBS], F32, tag="dens")
            nc.vector.tensor_copy(dens, xn[:, :, 0])

            # ---- row max over onehot block: TT-max folds (2x bf16) + reduce ----
            msc = app.tile([128, SUBS, 512], BF16, tag="msc")
            nc.vector.tensor_tensor(out=msc, in0=xn[:, :, 4:516],
                                    in1=xn[:, :, 516:1028], op=ALU.max)
            nc.vector.tensor_tensor(out=msc[:, :, 0:256], in0=msc[:, :, 0:256],
                                    in1=msc[:, :, 256:512], op=ALU.max)
            mx = sm.tile([128, SUBS], F32, tag="mx")
            nc.vector.reduce_max(out=mx, in_=msc[:, :, 0:256], axis=AX.X)

            # ---- A = (x == mx) * density ; A2 = A * rm ----
            A = app.tile([128, SUBS, T], BF16, tag="A")
            for s in range(SUBS):
                nc.vector.tensor_scalar(
                    out=A[:, s, :], in0=xn[:, s, 4:1028],
                    scalar1=mx[:, s:s + 1], scalar2=dens[:, s:s + 1],
                    op0=ALU.is_equal, op1=ALU.mult,
                )
            A2 = app.tile([128, SUBS, T], BF16, tag="A2")
            nc.vector.tensor_tensor(out=A2, in0=A, in1=rm_bc, op=ALU.mult)

            # ---- ce = sum_j A2 (one-hot -> exact): DMA add-tree + ACT accum ----
            nc.gpsimd.dma_start(out=A2[:, :, 0:512], in_=A2[:, :, 512:1024],
                                accum_op=ALU.add)
            nc.gpsimd.dma_start(out=A2[:, :, 0:256], in_=A2[:, :, 256:512],
                                accum_op=ALU.add)
            ce = sm.tile([128, SUBS], F32, tag="ce")
            trash = app.tile([128, SUBS, 256], BF16, tag="trash")
            for s in range(SUBS):
                nc.scalar.activation(out=trash[:, s, :], in_=A2[:, s, 0:256],
                                     func=AF.Copy, accum_out=ce[:, s:s + 1])

            # ---- ce flattened to one row (partition 0) via rearranging DMA ----
            # tile columns are ordered n = r*SUBS + s (r = partition of the
            # natural layout, s = sub-block), which is exactly ce's (r, s)
            # iteration order, so this DMA is a contiguous repack.
            cet = sm.tile([1, TILE], BF16, tag="cet")
            nc.gpsimd.dma_start(out=cet, in_=ce)

            # ---- h1T = W.T @ xT  (+ rank-1 ce ⊗ w1[4]) ----
            h1ps = ps_h1.tile([H1, TILE], F32, tag="h1ps")
            for c in range(8):
                nc.tensor.matmul(
                    h1ps, lhsT=wbig_sb[:, c, :],
                    rhs=xt[:, :, c, :].rearrange("p s r -> p r s"),
                    start=(c == 0), stop=False)
            nc.tensor.matmul(h1ps, lhsT=w9_sb,
                             rhs=xt9.rearrange("p s r -> p r s"),
                             start=False, stop=False)
            nc.tensor.matmul(h1ps, lhsT=w14_sb, rhs=cet, start=False,
                             stop=True)
            h1 = sm.tile([H1, TILE], BF16, tag="h1")
            nc.scalar.activation(out=h1, in_=h1ps, func=AF.Relu, bias=b1_sb,
                                 scale=1.0)

            # ---- h2, h3 ----
            h2ps = ps_sm.tile([H2, TILE], F32, tag="h2ps")
            nc.tensor.matmul(h2ps, lhsT=w2_sb, rhs=h1)
            h2 = sm.tile([H2, TILE], BF16, tag="h2")
            nc.scalar.activation(out=h2, in_=h2ps, func=AF.Relu, bias=b2_sb,
                                 scale=1.0)
            h3ps = ps_sm.tile([1, TILE], F32, tag="h3ps")
            nc.tensor.matmul(h3ps, lhsT=w3_sb, rhs=h2)
            osb = sm.tile([1, TILE], F32, tag="osb")
            nc.scalar.activation(out=osb, in_=h3ps, func=AF.Relu, bias=b3_sb,
                                 scale=1.0)

            # out column n = r*SUBS + s maps to row s*128 + r of this tile
            nc.sync.dma_start(
                out=out[t * TILE:(t + 1) * TILE].rearrange("(s r) -> r s", s=SUBS),
                in_=osb)

    return body


def host_prep(contention, w1, b1, w2, b2, w3, b3):
    """Build the small constant arrays shipped to every core."""
    bf = ml_dtypes.bfloat16
    f32 = np.float32
    rm = np.asarray(contention, dtype=f32).mean(axis=1)          # (T,)
    rm_b = np.tile(rm.astype(bf)[None, :], (128, 1))             # (128, T)

    w1 = np.asarray(w1, dtype=f32)
    # Wfull rows follow x columns 0..1027: col<4 -> w1[col]; col>=4 -> w1[col+1]
    wfull = np.concatenate([w1[0:4], w1[5:1029]], axis=0)        # (1028, H1)
    wbig = wfull[0:1024].reshape(8, 128, H1).astype(bf)          # cols 0:1024
    w9 = np.zeros((128, H1), dtype=f32)
    w9[124:128] = wfull[1024:1028]                               # cols 1024:1028
    w9 = w9.astype(bf)
    w14 = w1[4:5, :].astype(bf)                                  # (1, H1)

    return {
        "rm_b": np.ascontiguousarray(rm_b),
        "wbig": np.ascontiguousarray(wbig),
        "w9": np.ascontiguousarray(w9),
        "w14": np.ascontiguousarray(w14),
        "w2": np.asarray(w2, dtype=f32).astype(bf),
        "w3": np.asarray(w3, dtype=f32).astype(bf),
        "b1": np.asarray(b1, dtype=f32).reshape(H1, 1).copy(),
        "b2": np.asarray(b2, dtype=f32).reshape(H2, 1).copy(),
        "b3": np.asarray(b3, dtype=f32).reshape(1, 1).copy(),
        "ident": np.eye(128, dtype=f32),
    }


INPUT_SPECS = [
    ("x", (ROWS_PER_CORE, D), F32),
    ("rm_b", (128, T), BF16),
    ("wbig", (8, 128, H1), BF16),
    ("w9", (128, H1), BF16),
    ("w14", (1, H1), BF16),
    ("w2", (H1, H2), BF16),
    ("w3", (H2, 1), BF16),
    ("b1", (H1, 1), F32),
    ("b2", (H2, 1), F32),
    ("b3", (1, 1), F32),
    ("ident", (128, 128), F32),
]


@lru_cache(maxsize=1)
def _build_nc():
    nc = bacc.Bacc(
        "TRN2",
        target_bir_lowering=False,
        debug=False,
        enable_asserts=False,
        num_devices=N_CORES,
    )
    ins = {
        name: nc.dram_tensor(name, shape, dt, kind="ExternalInput").ap()
        for name, shape, dt in INPUT_SPECS
    }
    outs = {
        "out": nc.dram_tensor("out", (ROWS_PER_CORE,), F32,
                              kind="ExternalOutput").ap()
    }
    body = make_body(ROWS_PER_CORE)
    with tile.TileContext(nc) as tc:
        body(tc, outs, ins)
    nc.compile()
    return nc


def kernel(**inputs) -> np.ndarray:
    from concourse.bass_utils import run_bass_kernel_spmd

    x = np.asarray(inputs["x"], dtype=np.float32)
    consts = host_prep(
        inputs["contention"], inputs["w1"], inputs["b1"],
        inputs["w2"], inputs["b2"], inputs["w3"], inputs["b3"],
    )

    nc = _build_nc()
    in_maps = []
    for c in range(N_CORES):
        shard = np.ascontiguousarray(
            x[c * ROWS_PER_CORE:(c + 1) * ROWS_PER_CORE]
        )
        in_maps.append({"x": shard, **consts})

    res = run_bass_kernel_spmd(nc, in_maps, core_ids=list(range(N_CORES)))
    return np.concatenate([r["out"] for r in res.results]).astype(np.float32)


if __name__ == "__main__":
    rng = np.random.default_rng(0)
    demo = {
        "x": rng.standard_normal((B, D), dtype=np.float32),
        "contention": (rng.standard_normal((T, T)) * 0.1).astype(np.float32),
        "w1": (rng.standard_normal((D, H1)) / math.sqrt(D)).astype(np.float32),
        "b1": np.zeros(H1, np.float32),
        "w2": (rng.standard_normal((H1, H2)) / math.sqrt(H1)).astype(np.float32),
        "b2": np.zeros(H2, np.float32),
        "w3": (rng.standard_normal((H2, 1)) / math.sqrt(H2)).astype(np.float32),
        "b3": np.zeros(1, np.float32),
    }
    y = kernel(**demo)
    print("out", y.shape, y.dtype, y[:8])


# revision 13
# speedup vs baseline: 1.1484x; 1.1484x over previous
"""Trainium2 Bass kernel for nn_ContentionPredictor (embedding_lookup, memory-bound).

Computes, for each row of x (B=131072, D=1029):
    task_id = argmax(x[:, 4:1028]); ce = mean(contention[task_id]) * x[:, 0]
    out = relu(relu(relu([x[:,0:4], ce, x[:,4:1028]] @ w1 + b1) @ w2 + b2) @ w3 + b3)

Strategy (data-parallel over 8 NeuronCores, 16384 rows each):
  - SWDGE cast-DMA loads x fp32->bf16 in natural layout; HWDGE xbar DMA-transpose
    produces the K-on-partitions operand for the PE matmuls (w1 stationary).
  - argmax/gather is done without indices: DMA CCE-max tree folds the onehot
    block 1024->128 cols, DVE reduce_max gives the row max; a dual-op
    tensor_scalar builds mask*(density); multiplying by the broadcast row-means
    of contention and summing (DMA CCE-add folds + ACT accumulate) yields
    ce = density * rowmean[argmax] exactly (mask is one-hot).
  - MLP runs transposed (h1T = W.T @ xT) with relu+bias fused into the ACT
    PSUM evictions; ce enters h1 as a rank-1 PE update via a PE-transposed ce.
"""

import math
import os
from contextlib import ExitStack
from functools import lru_cache

import numpy as np
import ml_dtypes

import concourse.bass as bass
import concourse.bacc as bacc
import concourse.tile as tile
from concourse import mybir
from concourse._compat import with_exitstack

BF16 = mybir.dt.bfloat16
F32 = mybir.dt.float32

B = 131072
D = 1029
T = 1024
H1, H2 = 64, 32
N_CORES = 8
ROWS_PER_CORE = B // N_CORES  # 16384
TILE = 512                    # rows per pipeline tile
SUBS = TILE // 128            # 4 partition sub-blocks per tile

AX = mybir.AxisListType
ALU = mybir.AluOpType
AF = mybir.ActivationFunctionType


def make_body(rows_per_core):
    n_tiles = rows_per_core // TILE
    assert n_tiles * TILE == rows_per_core

    @with_exitstack
    def body(ctx: ExitStack, tc: tile.TileContext, outs, ins):
        nc = tc.nc
        x = ins["x"]
        out = outs["out"]

        singles = ctx.enter_context(tc.tile_pool(name="singles", bufs=1))
        xp = ctx.enter_context(tc.tile_pool(name="xp", bufs=3))
        xtp = ctx.enter_context(tc.tile_pool(name="xtp", bufs=3))
        app = ctx.enter_context(tc.tile_pool(name="app", bufs=3))
        sm = ctx.enter_context(tc.tile_pool(name="sm", bufs=4))
        ps_h1 = ctx.enter_context(tc.tile_pool(name="ps_h1", bufs=2, space="PSUM"))
        ps_sm = ctx.enter_context(tc.tile_pool(name="ps_sm", bufs=2, space="PSUM"))

        # --- constants (loaded once) ---
        rm_sb = singles.tile([128, T], BF16)
        nc.sync.dma_start(out=rm_sb, in_=ins["rm_b"])
        wbig_sb = singles.tile([128, 8, H1], BF16)
        nc.sync.dma_start(out=wbig_sb, in_=ins["wbig"].rearrange("c k m -> k c m"))
        w9_sb = singles.tile([128, H1], BF16)
        nc.sync.dma_start(out=w9_sb, in_=ins["w9"])
        w14_sb = singles.tile([1, H1], BF16)
        nc.sync.dma_start(out=w14_sb, in_=ins["w14"])
        w2_sb = singles.tile([H1, H2], BF16)
        nc.sync.dma_start(out=w2_sb, in_=ins["w2"])
        w3_sb = singles.tile([H2, 1], BF16)
        nc.sync.dma_start(out=w3_sb, in_=ins["w3"])
        b1_sb = singles.tile([H1, 1], F32)
        nc.sync.dma_start(out=b1_sb, in_=ins["b1"])
        b2_sb = singles.tile([H2, 1], F32)
        nc.sync.dma_start(out=b2_sb, in_=ins["b2"])
        b3_sb = singles.tile([1, 1], F32)
        nc.sync.dma_start(out=b3_sb, in_=ins["b3"])
        id_sb = singles.tile([128, 128], F32)
        nc.sync.dma_start(out=id_sb, in_=ins["ident"])

        # rm broadcast across the sub-block (mid) dim: (128, SUBS, 1024), step 0
        rm_ap = rm_sb[:, :]
        rm_bc = bass.AP(
            tensor=rm_ap.tensor,
            offset=rm_ap.offset,
            ap=[rm_ap.ap[0], [0, SUBS], rm_ap.ap[1]],
        )

        for t in range(n_tiles):
            # ---- load tile (cast fp32 -> bf16), natural layout ----
            xr = x[t * TILE:(t + 1) * TILE, 0:1028].rearrange(
                "(s p) c -> p s c", p=128
            )
            xn = xp.tile([128, SUBS, 1028], BF16, tag="xn")
            nc.gpsimd.dma_start(out=xn, in_=xr)

            # ---- transpose for matmul: cols 0:1024 (8 chunks) + tail 900:1028 ----
            xt = xtp.tile([128, SUBS, 8, 128], BF16, tag="xt")
            xt9 = xtp.tile([128, SUBS, 128], BF16, tag="xt9")
            for s in range(SUBS):
                eng = nc.sync if s % 2 == 0 else nc.scalar
                eng.dma_start(out=xt[:, s, :, :], in_=xn[:, s, 0:1024],
                              transpose=True)
                eng2 = nc.scalar if s % 2 == 0 else nc.sync
                eng2.dma_start(out=xt9[:, s, :], in_=xn[:, s, 900:1028],
                               transpose=True)

            # ---- density (col 0) as fp32 per-partition scalars ----
            dens = sm.tile([128, SUBS], F32, tag="dens")
            nc.vector.tensor_copy(dens, xn[:, :, 0])

            # ---- row max over onehot block: TT-max folds (2x bf16) + reduce ----
            msc = app.tile([128, SUBS, 512], BF16, tag="msc")
            nc.vector.tensor_tensor(out=msc, in0=xn[:, :, 4:516],
                                    in1=xn[:, :, 516:1028], op=ALU.max)
            nc.vector.tensor_tensor(out=msc[:, :, 0:256], in0=msc[:, :, 0:256],
                                    in1=msc[:, :, 256:512], op=ALU.max)
            mx = sm.tile([128, SUBS], F32, tag="mx")
            nc.vector.reduce_max(out=mx, in_=msc[:, :, 0:256], axis=AX.X)

            # ---- A = (x == mx) * density ; A2 = A * rm ----
            A = app.tile([128, SUBS, T], BF16, tag="A")
            for s in range(SUBS):
                nc.vector.tensor_scalar(
                    out=A[:, s, :], in0=xn[:, s, 4:1028],
                    scalar1=mx[:, s:s + 1], scalar2=dens[:, s:s + 1],
                    op0=ALU.is_equal, op1=ALU.mult,
                )
            A2 = app.tile([128, SUBS, T], BF16, tag="A2")
            nc.vector.tensor_tensor(out=A2, in0=A, in1=rm_bc, op=ALU.mult)

            # ---- ce = sum_j A2 (one-hot -> exact): DMA add-tree + ACT accum ----
            nc.gpsimd.dma_start(out=A2[:, :, 0:512], in_=A2[:, :, 512:1024],
                                accum_op=ALU.add)
            nc.gpsimd.dma_start(out=A2[:, :, 0:256], in_=A2[:, :, 256:512],
                                accum_op=ALU.add)
            ce = sm.tile([128, SUBS], F32, tag="ce")
            trash = app.tile([128, SUBS, 256], BF16, tag="trash")
            for s in range(SUBS):
                nc.scalar.activation(out=trash[:, s, :], in_=A2[:, s, 0:256],
                                     func=AF.Copy, accum_out=ce[:, s:s + 1])

            # ---- ce flattened to one row (partition 0): one DMA per sub ----
            cet = sm.tile([1, TILE], BF16, tag="cet")
            for s in range(SUBS):
                nc.gpsimd.dma_start(out=cet[0:1, s * 128:(s + 1) * 128],
                                    in_=ce[:, s:s + 1])

            # ---- h1T = W.T @ xT  (+ rank-1 ce ⊗ w1[4]) ----
            h1ps = ps_h1.tile([H1, TILE], F32, tag="h1ps")
            for c in range(8):
                nc.tensor.matmul(h1ps, lhsT=wbig_sb[:, c, :], rhs=xt[:, :, c, :],
                                 start=(c == 0), stop=False)
            nc.tensor.matmul(h1ps, lhsT=w9_sb, rhs=xt9, start=False, stop=False)
            nc.tensor.matmul(h1ps, lhsT=w14_sb, rhs=cet, start=False,
                             stop=True)
            h1 = sm.tile([H1, TILE], BF16, tag="h1")
            nc.scalar.activation(out=h1, in_=h1ps, func=AF.Relu, bias=b1_sb,
                                 scale=1.0)

            # ---- h2, h3 ----
            h2ps = ps_sm.tile([H2, TILE], F32, tag="h2ps")
            nc.tensor.matmul(h2ps, lhsT=w2_sb, rhs=h1)
            h2 = sm.tile([H2, TILE], BF16, tag="h2")
            nc.scalar.activation(out=h2, in_=h2ps, func=AF.Relu, bias=b2_sb,
                                 scale=1.0)
            h3ps = ps_sm.tile([1, TILE], F32, tag="h3ps")
            nc.tensor.matmul(h3ps, lhsT=w3_sb, rhs=h2)
            osb = sm.tile([1, TILE], F32, tag="osb")
            nc.scalar.activation(out=osb, in_=h3ps, func=AF.Relu, bias=b3_sb,
                                 scale=1.0)

            nc.sync.dma_start(out=out[t * TILE:(t + 1) * TILE], in_=osb)

    return body


def host_prep(contention, w1, b1, w2, b2, w3, b3):
    """Build the small constant arrays shipped to every core."""
    bf = ml_dtypes.bfloat16
    f32 = np.float32
    rm = np.asarray(contention, dtype=f32).mean(axis=1)          # (T,)
    rm_b = np.tile(rm.astype(bf)[None, :], (128, 1))             # (128, T)

    w1 = np.asarray(w1, dtype=f32)
    # Wfull rows follow x columns 0..1027: col<4 -> w1[col]; col>=4 -> w1[col+1]
    wfull = np.concatenate([w1[0:4], w1[5:1029]], axis=0)        # (1028, H1)
    wbig = wfull[0:1024].reshape(8, 128, H1).astype(bf)          # cols 0:1024
    w9 = np.zeros((128, H1), dtype=f32)
    w9[124:128] = wfull[1024:1028]                               # cols 1024:1028
    w9 = w9.astype(bf)
    w14 = w1[4:5, :].astype(bf)                                  # (1, H1)

    return {
        "rm_b": np.ascontiguousarray(rm_b),
        "wbig": np.ascontiguousarray(wbig),
        "w9": np.ascontiguousarray(w9),
        "w14": np.ascontiguousarray(w14),
        "w2": np.asarray(w2, dtype=f32).astype(bf),
        "w3": np.asarray(w3, dtype=f32).astype(bf),
        "b1": np.asarray(b1, dtype=f32).reshape(H1, 1).copy(),
        "b2": np.asarray(b2, dtype=f32).reshape(H2, 1).copy(),
        "b3": np.asarray(b3, dtype=f32).reshape(1, 1).copy(),
        "ident": np.eye(128, dtype=f32),
    }


INPUT_SPECS = [
    ("x", (ROWS_PER_CORE, D), F32),
    ("rm_b", (128, T), BF16),
    ("wbig", (8, 128, H1), BF16),
    ("w9", (128, H1), BF16),
    ("w14", (1, H1), BF16),
    ("w2", (H1, H2), BF16),
    ("w3", (H2, 1), BF16),
    ("b1", (H1, 1), F32),
    ("b2", (H2, 1), F32),
    ("b3", (1, 1), F32),
    ("ident", (128, 128), F32),
]


@lru_cache(maxsize=1)
def _build_nc():
    nc = bacc.Bacc(
        "TRN2",
        target_bir_lowering=False,
        debug=False,
        enable_asserts=False,
        num_devices=N_CORES,
    )
    ins = {
        name: nc.dram_tensor(name, shape, dt, kind="ExternalInput").ap()
        for name, shape, dt in INPUT_SPECS
    }
    outs = {
        "out": nc.dram_tensor("out", (ROWS_PER_CORE,), F32,
                              kind="ExternalOutput").ap()
    }
    body = make_body(ROWS_PER_CORE)
    with tile.TileContext(nc) as tc:
        body(tc, outs, ins)
    nc.compile()
    return nc


def kernel(**inputs) -> np.ndarray:
    from concourse.bass_utils import run_bass_kernel_spmd

    x = np.asarray(inputs["x"], dtype=np.float32)
    consts = host_prep(
        inputs["contention"], inputs["w1"], inputs["b1"],
        inputs["w2"], inputs["b2"], inputs["w3"], inputs["b3"],
    )

    nc = _build_nc()
    in_maps = []
    for c in range(N_CORES):
        shard = np.ascontiguousarray(
            x[c * ROWS_PER_CORE:(c + 1) * ROWS_PER_CORE]
        )
        in_maps.append({"x": shard, **consts})

    res = run_bass_kernel_spmd(nc, in_maps, core_ids=list(range(N_CORES)))
    return np.concatenate([r["out"] for r in res.results]).astype(np.float32)


if __name__ == "__main__":
    rng = np.random.default_rng(0)
    demo = {
        "x": rng.standard_normal((B, D), dtype=np.float32),
        "contention": (rng.standard_normal((T, T)) * 0.1).astype(np.float32),
        "w1": (rng.standard_normal((D, H1)) / math.sqrt(D)).astype(np.float32),
        "b1": np.zeros(H1, np.float32),
        "w2": (rng.standard_normal((H1, H2)) / math.sqrt(H1)).astype(np.float32),
        "b2": np.zeros(H2, np.float32),
        "w3": (rng.standard_normal((H2, 1)) / math.sqrt(H2)).astype(np.float32),
        "b3": np.zeros(1, np.float32),
    }
    y = kernel(**demo)
    print("out", y.shape, y.dtype, y[:8])
